# revision 7
# baseline (speedup 1.0000x reference)
"""Trainium2 Bass kernel: 16-head attention with RoPE (dense_transformer).

Sharding: tensor-parallel over heads. 8 cores x 2 heads each.
Each core: Wq/Wk/Wv column slice [1024,128], Wo row slice [128,1024],
full input; computes its heads' attention + partial output projection.
Host sums the 8 partial outputs (row-parallel Wo reduction) and adds bo.

Device layout is "transposed": Q^T/K^T/ctx^T are kept as [dim, seq] with
the head dim on SBUF partitions, so Q^T = Wq^T @ X^T comes straight out
of the PE, scores^T = K^T.T @ Q^T needs no transposes, and the softmax
denominator falls out of an extra ones-column appended to V.

v2 schedule: the kernel is paced by the softmax exp (ACT engine is the
only engine with an exp LUT; 16.8M elements/core ~= 143us at 1
elem/lane/cycle). Everything else hides around it:
 - xt arrives as 8 channel-major chunks (8KB DMA lines); Q/K run
   ch-outer over 8 PSUM banks so the first matmul only needs chunk 0.
 - V-projection matmuls are issued between the first two attention
   blocks' scores and ctx, filling the PE while ACT runs exp.
 - batch-0 output-projection matmuls are issued as fillers inside
   batch-1 attention blocks; their PSUM->SBUF copies stay off ACT.
 - softmax denominators collect in a [16,512] tile, reciprocal on DVE,
   one DRAM bounce for the partition broadcast, per-(b,h) so batch-0
   divisions overlap batch-1 attention.
"""

import sys

if "/opt/trn_rl_repo" not in sys.path:
    sys.path.insert(0, "/opt/trn_rl_repo")

import numpy as np
import ml_dtypes

B = 2
S = 2048
NS = B * S  # 4096
D = 1024
H = 16
DK = 64
NCORES = 8
HPC = H // NCORES  # heads per core = 2
DPC = HPC * DK  # model dims per core = 128

_cache = {}


def _build_nc():
    import concourse.bass as bass
    import concourse.tile as tile
    import concourse.mybir as mybir
    from concourse import bacc

    fp32 = mybir.dt.float32
    bf16 = mybir.dt.bfloat16
    Exp = mybir.ActivationFunctionType.Exp

    nc = bacc.Bacc("TRN2", debug=False, num_devices=NCORES)

    xt = nc.dram_tensor("xt", [D, NS], bf16, kind="ExternalInput").ap()
    wq = nc.dram_tensor("wq", [128, 8 * 128], bf16, kind="ExternalInput").ap()
    wk = nc.dram_tensor("wk", [128, 8 * 128], bf16, kind="ExternalInput").ap()
    wv = nc.dram_tensor("wv", [128, 8 * 128], bf16, kind="ExternalInput").ap()
    wo = nc.dram_tensor("wo", [DPC, D], bf16, kind="ExternalInput").ap()
    bq = nc.dram_tensor("bq", [DPC, 1], fp32, kind="ExternalInput").ap()
    bk = nc.dram_tensor("bk", [DPC, 1], fp32, kind="ExternalInput").ap()
    bv = nc.dram_tensor("bv", [1, DPC], bf16, kind="ExternalInput").ap()
    cos_d = nc.dram_tensor("cos", [128, S], bf16, kind="ExternalInput").ap()
    sin_d = nc.dram_tensor("sin", [128, S], bf16, kind="ExternalInput").ap()
    out_d = nc.dram_tensor("out", [D, NS], bf16, kind="ExternalOutput").ap()

    with tile.TileContext(nc) as tc:
        with (
            tc.tile_pool(name="persist", bufs=1) as persist,
            tc.tile_pool(name="dram", bufs=1, space="DRAM") as dram,
        ):
            qrot = persist.tile([128, NS], bf16, tag="qrot")
            krot = persist.tile([128, NS], bf16, tag="krot")
            # v_sb[:, tt, 65h : 65h+64] = V rows tt*128.. for head h,
            # v_sb[:, tt, 65h+64] = 1.0 (denominator column)
            v_sb = persist.tile([128, 32, 2 * (DK + 1)], bf16, tag="v")
            ctxT = persist.tile([128, NS], bf16, tag="ctxT")
            wo_sb = persist.tile([128, 8, 128], bf16, tag="wo")
            # softmax denominators: row 32*(b*2+h) holds that head's [S]
            # den chunks (st_i*512..); 32-aligned bases keep TensorCopy legal
            den_big = persist.tile([128, S], fp32, tag="den_big")
            rec_big = persist.tile([128, S], bf16, tag="rec_big")
            rec_dram = dram.tile([16 * 512], bf16, tag="rec_dram")

            # ---------------- Phase 1: QKV projections + RoPE ----------------
            # qkv_sb (xt + V weights) must outlive phase 1: the V-projection
            # runs inside the attention phase as PE filler work. Enter it on
            # an ExitStack closed after attention.
            from contextlib import ExitStack

            _qkv_stk = ExitStack()
            qkv_sb = _qkv_stk.enter_context(tc.tile_pool(name="qkv_sb", bufs=1))
            with (
                tc.tile_pool(name="qkv_tmp", bufs=2) as qkv_tmp,
                tc.tile_pool(name="qkv_ps", bufs=1, space="PSUM") as qkv_ps,
            ):
                cos_sb = qkv_sb.tile([128, S], bf16, tag="cos")
                sin_sb = qkv_sb.tile([128, S], bf16, tag="sin")
                wq_sb = qkv_sb.tile([128, 8, 128], bf16, tag="wq")
                wk_sb = qkv_sb.tile([128, 8, 128], bf16, tag="wk")
                wv_sb = qkv_sb.tile([128, 8, 128], bf16, tag="wv")
                bq_sb = qkv_sb.tile([128, 1], fp32, tag="bq")
                bk_sb = qkv_sb.tile([128, 1], fp32, tag="bk")
                bvb = qkv_sb.tile([128, 128], bf16, tag="bvb")
                xt_sb = qkv_sb.tile([128, 8, NS], bf16, tag="xt")
                xt_r = xt.rearrange("(c p) s -> p c s", p=128)

                # DMA plan: sync gets wq + even xt chunks (Q needs chunk c
                # at step c); gpsimd gets wk + odd chunks + wv/cos/sin;
                # vector queue gets the small tail.
                nc.sync.dma_start(wq_sb[:], wq.rearrange("p (c m) -> p c m", m=128))
                nc.gpsimd.dma_start(wk_sb[:], wk.rearrange("p (c m) -> p c m", m=128))
                for c in range(8):
                    eng = nc.sync if c % 2 == 0 else nc.gpsimd
                    eng.dma_start(xt_sb[:, c : c + 1, :], xt_r[:, c : c + 1, :])
                nc.gpsimd.dma_start(wv_sb[:], wv.rearrange("p (c m) -> p c m", m=128))
                nc.gpsimd.dma_start(cos_sb[:], cos_d)
                nc.gpsimd.dma_start(sin_sb[:], sin_d)
                nc.scalar.dma_start(bq_sb[:], bq)
                nc.scalar.dma_start(bk_sb[:], bk)
                nc.scalar.dma_start(bvb[:], bv.to_broadcast((128, 128)))
                nc.scalar.dma_start(wo_sb[:], wo.rearrange("p (j m) -> p j m", m=128))

                # Q and K: out[d, s] (d on partitions). ch-outer over 8 PSUM
                # banks: the first 8 matmuls need only xt chunk 0, so the PE
                # starts ~3us in instead of waiting for the full 8MB of xt.
                for w_sb, b_sb, rot in ((wq_sb, bq_sb, qrot), (wk_sb, bk_sb, krot)):
                    plain = qkv_tmp.tile([128, NS], bf16, tag="plain")
                    swap = qkv_tmp.tile([128, NS], bf16, tag="swap")
                    ps = [
                        qkv_ps.tile([128, 512], fp32, tag=f"b{st}", name=f"ps{st}")
                        for st in range(8)
                    ]
                    for ch in range(8):
                        for st in range(8):
                            nc.tensor.matmul(
                                ps[st][:],
                                w_sb[:, ch, :],
                                xt_sb[:, ch, st * 512 : (st + 1) * 512],
                                start=(ch == 0),
                                stop=(ch == 7),
                            )
                    for st in range(8):
                        nc.vector.tensor_scalar_add(
                            plain[:, st * 512 : (st + 1) * 512], ps[st][:], b_sb[:]
                        )
                    # swap rope halves within each head (cross-partition)
                    for g in (0, 64):
                        nc.scalar.dma_start(
                            swap[g : g + 32, :], plain[g + 32 : g + 64, :]
                        )
                        nc.scalar.dma_start(
                            swap[g + 32 : g + 64, :], plain[g : g + 32, :]
                        )
                    for b in range(B):
                        sl = slice(b * S, (b + 1) * S)
                        tmp = qkv_tmp.tile([128, S], bf16, tag="ropetmp")
                        nc.vector.tensor_mul(rot[:, sl], plain[:, sl], cos_sb[:])
                        nc.vector.tensor_mul(tmp[:], swap[:, sl], sin_sb[:])
                        nc.vector.tensor_add(rot[:, sl], rot[:, sl], tmp[:])

                ones_ap = v_sb[:].rearrange("p t (h x) -> p t h x", x=DK + 1)[
                    :, :, :, DK
                ]
                nc.vector.memset(ones_ap, 1.0)

            # -------- Phase 2: attention (V-proj + b0 out-proj as fillers) ----
            with (
                tc.tile_pool(name="att_sb", bufs=2) as att_sb,
                tc.tile_pool(name="sc_ps", bufs=2, space="PSUM") as sc_ps,
                tc.tile_pool(name="ctx_ps", bufs=2, space="PSUM") as ctx_ps,
                tc.tile_pool(name="op_ps", bufs=2, space="PSUM") as op_ps,
                tc.tile_pool(name="op_sb", bufs=4) as op_sb,
            ):

                def v_proj(half):
                    # V for batch `half` in [t, dk] layout; uses op_ps banks
                    # (free until the b1 out-proj fillers start).
                    for tt in range(half * 16, half * 16 + 16):
                        psv = op_ps.tile(
                            [128, 128], fp32, tag="op", name=f"psv{tt}"
                        )
                        for ch in range(8):
                            nc.tensor.matmul(
                                psv[:],
                                xt_sb[:, ch, tt * 128 : (tt + 1) * 128],
                                wv_sb[:, ch, :],
                                start=(ch == 0),
                                stop=(ch == 7),
                            )
                        dst = v_sb[:, tt].rearrange("p (h x) -> p h x", h=2)[
                            :, :, 0:DK
                        ]
                        nc.vector.tensor_add(dst, psv[:], bvb[:])

                def op_tile(st, oc, copy_eng):
                    po = op_ps.tile([128, 512], fp32, tag="op", name=f"po{st}_{oc}")
                    nc.tensor.matmul(
                        po[:],
                        wo_sb[:, oc, :],
                        ctxT[:, st * 512 : (st + 1) * 512],
                        start=True,
                        stop=True,
                    )
                    ob = op_sb.tile([128, 512], bf16, tag="ob", name=f"ob{st}_{oc}")
                    copy_eng(ob[:], po[:])
                    nc.sync.dma_start(
                        out_d[oc * 128 : (oc + 1) * 128, st * 512 : (st + 1) * 512],
                        ob[:],
                    )

                def attn_block(b, h, sh, pre_ctx=None, fillers=None):
                    pi = b * HPC + h
                    hh = h * DK
                    expS = att_sb.tile(
                        [128, 16, 1024], bf16, tag="expS", name=f"expS{pi}_{sh}"
                    )
                    for tt in range(16):
                        ps = sc_ps.tile([128, 1024], fp32, tag="sc", name="scps")
                        for si in range(2):
                            s0 = b * S + sh * 1024 + si * 512
                            nc.tensor.matmul(
                                ps[:, si * 512 : (si + 1) * 512],
                                krot[
                                    hh : hh + DK,
                                    b * S + tt * 128 : b * S + (tt + 1) * 128,
                                ],
                                qrot[hh : hh + DK, s0 : s0 + 512],
                                start=True,
                                stop=True,
                            )
                        nc.scalar.activation(expS[:, tt, :], ps[:], Exp, scale=0.125)
                        if fillers is not None:
                            f = next(fillers, None)
                            if f is not None:
                                f()
                    if pre_ctx is not None:
                        pre_ctx()
                    for sq in range(2):  # 512-wide ctx tiles
                        st_i = sh * 2 + sq
                        pc = ctx_ps.tile([DK + 1, 512], fp32, tag="pc", name="pcps")
                        for tt in range(16):
                            nc.tensor.matmul(
                                pc[:],
                                v_sb[
                                    :,
                                    b * 16 + tt,
                                    h * (DK + 1) : (h + 1) * (DK + 1),
                                ],
                                expS[:, tt, sq * 512 : (sq + 1) * 512],
                                start=(tt == 0),
                                stop=(tt == 15),
                            )
                        ds0 = b * S + st_i * 512
                        if h == 0:
                            nc.vector.tensor_copy(
                                ctxT[0:DK, ds0 : ds0 + 512], pc[0:DK, :]
                            )
                        else:
                            stg = att_sb.tile([DK, 512], bf16, tag="stg", name="stg")
                            nc.vector.tensor_copy(stg[:], pc[0:DK, :])
                            nc.vector.stream_shuffle(
                                ctxT[DK : 2 * DK, ds0 : ds0 + 512],
                                stg[:],
                                mask=list(range(32)),
                            )
                        dr = 32 * pi
                        nc.vector.tensor_copy(
                            den_big[dr : dr + 1, st_i * 512 : (st_i + 1) * 512],
                            pc[DK : DK + 1, :],
                        )
                        with nc.allow_low_precision(
                            reason="bf16 softmax reciprocal within tolerance"
                        ):
                            nc.vector.reciprocal(
                                rec_big[dr : dr + 1, st_i * 512 : (st_i + 1) * 512],
                                den_big[dr : dr + 1, st_i * 512 : (st_i + 1) * 512],
                            )

                def div_head(b, h):
                    # per-chunk reciprocals already landed in rec_big; bounce
                    # through DRAM to broadcast along partitions, then scale
                    # this head's ctxT rows.
                    pi = b * HPC + h
                    hh = h * DK
                    dr = 32 * pi
                    nc.gpsimd.dma_start(
                        rec_dram[pi * S : (pi + 1) * S].rearrange(
                            "(o c) -> o c", o=1
                        ),
                        rec_big[dr : dr + 1, :],
                    )
                    R = att_sb.tile([128, S], bf16, tag="R", name=f"R{pi}")
                    nc.gpsimd.dma_start(
                        R[hh : hh + DK, :],
                        rec_dram[pi * S : (pi + 1) * S]
                        .rearrange("(o s) -> o s", o=1)
                        .to_broadcast((DK, S)),
                    )
                    sl = slice(b * S, (b + 1) * S)
                    nc.vector.tensor_mul(
                        ctxT[hh : hh + DK, sl],
                        ctxT[hh : hh + DK, sl],
                        R[hh : hh + DK, :],
                    )

                # batch 0 blocks; V-proj hides under the first two blocks' exp
                attn_block(0, 0, 0, pre_ctx=lambda: v_proj(0))
                attn_block(0, 0, 1, pre_ctx=lambda: v_proj(1))
                attn_block(0, 1, 0)
                attn_block(0, 1, 1)
                div_head(0, 0)
                div_head(0, 1)

                # batch 1 blocks; batch-0 out-proj rides as fillers (skip the
                # first block so the b0 divisions are done before the first
                # op matmul reaches the PE queue head)
                op0 = iter(
                    [
                        (lambda st_, oc_: lambda: op_tile(st_, oc_, nc.vector.tensor_copy))(
                            st, oc
                        )
                        for st in range(4)
                        for oc in range(8)
                    ]
                )
                attn_block(1, 0, 0)
                attn_block(1, 0, 1, fillers=op0)
                div_head(1, 0)
                attn_block(1, 1, 0, fillers=op0)
                attn_block(1, 1, 1, fillers=op0)
                div_head(1, 1)

                # ---------------- Phase 3: batch-1 output projection ---------
                for st in range(4, 8):
                    for oc in range(8):
                        copy_eng = (
                            nc.vector.tensor_copy
                            if (st * 8 + oc) % 2 == 0
                            else nc.scalar.copy
                        )
                        op_tile(st, oc, copy_eng)

            _qkv_stk.close()

    nc.compile()
    return nc


def _rope_tables():
    pos = np.arange(S, dtype=np.float64)
    inv_freq = np.exp(np.arange(0, DK, 2, dtype=np.float64) * (-np.log(10000.0) / DK))
    ang = pos[:, None] * inv_freq[None, :]  # [S, 32]
    cos_t = np.empty((128, S), dtype=np.float32)
    sin_t = np.empty((128, S), dtype=np.float32)
    c = np.cos(ang).astype(np.float32).T  # [32, S]
    s = np.sin(ang).astype(np.float32).T
    for blk in range(4):
        cos_t[blk * 32 : (blk + 1) * 32] = c
        sign = -1.0 if blk % 2 == 0 else 1.0
        sin_t[blk * 32 : (blk + 1) * 32] = sign * s
    return cos_t, sin_t


def _prep_w(w):
    # [1024, 128] column slice -> [128, 8*128] with the 1024-dim split into
    # 8 chunks of 128 on the partition axis (contiguous 2KB DMA lines)
    bf = ml_dtypes.bfloat16
    return np.ascontiguousarray(
        np.asarray(w, dtype=np.float32)
        .reshape(8, 128, 128)
        .transpose(1, 0, 2)
        .reshape(128, 8 * 128)
    ).astype(bf)


def _prep_inputs(inputs, Wq, bq, Wk, bk, Wv, bv, Wo):
    bf = ml_dtypes.bfloat16
    x2 = np.asarray(inputs, dtype=np.float32).reshape(NS, D)
    xt = np.ascontiguousarray(x2.T).astype(bf)
    cos_t, sin_t = _rope_tables()
    cos_b = cos_t.astype(bf)
    sin_b = sin_t.astype(bf)
    in_maps = []
    for c in range(NCORES):
        sl = slice(c * DPC, (c + 1) * DPC)
        in_maps.append(
            {
                "xt": xt,
                "wq": _prep_w(Wq[:, sl]),
                "wk": _prep_w(Wk[:, sl]),
                "wv": _prep_w(Wv[:, sl]),
                "wo": np.ascontiguousarray(Wo[sl, :]).astype(bf),
                "bq": np.ascontiguousarray(bq[sl]).reshape(DPC, 1).astype(np.float32),
                "bk": np.ascontiguousarray(bk[sl]).reshape(DPC, 1).astype(np.float32),
                "bv": np.ascontiguousarray(bv[sl]).reshape(1, DPC).astype(bf),
                "cos": cos_b,
                "sin": sin_b,
            }
        )
    return in_maps


def _get_nc():
    if "nc" not in _cache:
        _cache["nc"] = _build_nc()
    return _cache["nc"]


def run(inputs_dict, trace=False):
    """Build (cached), run on 8 cores, assemble full output. Returns
    (output fp32 [B,S,D], BassKernelResults)."""
    from concourse.bass_utils import run_bass_kernel_spmd

    nc = _get_nc()
    in_maps = _prep_inputs(
        inputs_dict["inputs"],
        inputs_dict["Wq"],
        inputs_dict["bq"],
        inputs_dict["Wk"],
        inputs_dict["bk"],
        inputs_dict["Wv"],
        inputs_dict["bv"],
        inputs_dict["Wo"],
    )
    res = run_bass_kernel_spmd(
        nc, in_maps, core_ids=list(range(NCORES)), trace=trace
    )
    acc = np.zeros((D, NS), dtype=np.float32)
    for r in res.results:
        acc += r["out"].astype(np.float32)
    out = acc.T.reshape(B, S, D) + np.asarray(inputs_dict["bo"], dtype=np.float32)
    return out.astype(np.float32), res


def kernel(**inputs):
    out, _ = run(inputs, trace=False)
    return out


# revision 11
# speedup vs baseline: 1.1053x; 1.1053x over previous
"""Trainium2 Bass kernel: 16-head attention with RoPE (dense_transformer).

Sharding: tensor-parallel over heads. 8 cores x 2 heads each.
Each core: Wq/Wk/Wv column slice [1024,128], Wo row slice [128,1024],
full input; computes its heads' attention + partial output projection.
Host sums the 8 partial outputs (row-parallel Wo reduction) and adds bo.

Device layout is "transposed": Q^T/K^T/ctx^T are kept as [dim, seq] with
the head dim on SBUF partitions, so Q^T = Wq^T @ X^T comes straight out
of the PE, scores^T = K^T.T @ Q^T needs no transposes, and the softmax
denominator falls out of an extra ones-column appended to V.

v3 schedule: the kernel is paced by the softmax exp (only the ACT
engine has an exp LUT; 16.8M elements/core ~= 146us at 1 elem/lane/
cycle). Everything else hides in its shadow:
 - xt arrives as 8 channel-major chunks (8KB DMA lines); Q/K chains run
   ch-outer per chunk, batch-0 halves first, so rope(b0) starts ~24us
   and the first exp fires ~33us (vs 91us originally).
 - scores for the two heads issue back-to-back as concurrent PE
   row-group tiles (0,0)/(64,0) - half the PE time of serial heads.
 - each block's ctx chains ride the NEXT block's exp shadow, as do the
   V-projection chains and batch-0's output projection ("pump" units).
 - softmax reciprocals batch 2048-at-a-time through a DRAM-bounce
   reshape to [128,16] (142ns) instead of per-chunk scalar ops.
"""

import sys

if "/opt/trn_rl_repo" not in sys.path:
    sys.path.insert(0, "/opt/trn_rl_repo")

from collections import deque

import numpy as np
import ml_dtypes

B = 2
S = 2048
NS = B * S  # 4096
D = 1024
H = 16
DK = 64
NCORES = 8
HPC = H // NCORES  # heads per core = 2
DPC = HPC * DK  # model dims per core = 128

_cache = {}


def _build_nc():
    import concourse.bass as bass
    import concourse.tile as tile
    import concourse.mybir as mybir
    from concourse import bacc

    fp32 = mybir.dt.float32
    bf16 = mybir.dt.bfloat16
    Exp = mybir.ActivationFunctionType.Exp

    nc = bacc.Bacc("TRN2", debug=False, num_devices=NCORES)

    xt = nc.dram_tensor("xt", [D, NS], bf16, kind="ExternalInput").ap()
    wq = nc.dram_tensor("wq", [128, 8 * 128], bf16, kind="ExternalInput").ap()
    wk = nc.dram_tensor("wk", [128, 8 * 128], bf16, kind="ExternalInput").ap()
    wv = nc.dram_tensor("wv", [128, 8 * 128], bf16, kind="ExternalInput").ap()
    wo = nc.dram_tensor("wo", [DPC, D], bf16, kind="ExternalInput").ap()
    bq = nc.dram_tensor("bq", [DPC, 1], fp32, kind="ExternalInput").ap()
    bk = nc.dram_tensor("bk", [DPC, 1], fp32, kind="ExternalInput").ap()
    bv = nc.dram_tensor("bv", [1, DPC], bf16, kind="ExternalInput").ap()
    cos_d = nc.dram_tensor("cos", [128, S], bf16, kind="ExternalInput").ap()
    sin_d = nc.dram_tensor("sin", [128, S], bf16, kind="ExternalInput").ap()
    out_d = nc.dram_tensor("out", [D, NS], bf16, kind="ExternalOutput").ap()

    with tile.TileContext(nc) as tc:
        with (
            tc.tile_pool(name="persist", bufs=1) as persist,
            tc.tile_pool(name="qkv_sb", bufs=1) as qkv_sb,
            tc.tile_pool(name="qkv_tmp", bufs=2) as qkv_tmp,
            tc.tile_pool(name="att_sb", bufs=1) as att_sb,
            tc.tile_pool(name="op_sb", bufs=2) as op_sb,
            tc.tile_pool(name="dram", bufs=1, space="DRAM") as dram,
        ):
            qrot = persist.tile([128, NS], bf16, tag="qrot")
            krot = persist.tile([128, NS], bf16, tag="krot")
            # v_sb[:, tt, 65h : 65h+64] = V rows tt*128.. for head h,
            # v_sb[:, tt, 65h+64] = 1.0 (denominator column)
            v_sb = persist.tile([128, 32, 2 * (DK + 1)], bf16, tag="v")
            ctxT = persist.tile([128, NS], bf16, tag="ctxT")
            wo_sb = persist.tile([128, 8, 128], bf16, tag="wo")
            # den chunks land on 32-aligned rows: row 32*(b*2+h), cols
            # st_i*512.. (TensorCopy across partitions needs 32-aligned
            # bases; tensor_tensor ops need exact matches)
            den_big = persist.tile([128, S], fp32, tag="den_big")
            dn128 = persist.tile([128, 4, 16], fp32, tag="dn128")
            rc128 = persist.tile([128, 4, 16], bf16, tag="rc128")
            den_dram = dram.tile([4 * S], fp32, tag="den_dram")
            rec_dram = dram.tile([4 * S], bf16, tag="rec_dram")

            cos_sb = qkv_sb.tile([128, S], bf16, tag="cos")
            sin_sb = qkv_sb.tile([128, S], bf16, tag="sin")
            wq_sb = qkv_sb.tile([128, 8, 128], bf16, tag="wq")
            wk_sb = qkv_sb.tile([128, 8, 128], bf16, tag="wk")
            wv_sb = qkv_sb.tile([128, 8, 128], bf16, tag="wv")
            bq_sb = qkv_sb.tile([128, 1], fp32, tag="bq")
            bk_sb = qkv_sb.tile([128, 1], fp32, tag="bk")
            bvb = qkv_sb.tile([128, 128], bf16, tag="bvb")
            xt_sb = qkv_sb.tile([128, 8, NS], bf16, tag="xt")
            xt_r = xt.rearrange("(c p) s -> p c s", p=128)

            # DMA plan: 3 queues. sync: wq + even chunks; gpsimd: wk + odd
            # chunks + wv/cos/sin; scalar(ACT, idle now): small tail.
            nc.sync.dma_start(wq_sb[:], wq.rearrange("p (c m) -> p c m", m=128))
            nc.gpsimd.dma_start(wk_sb[:], wk.rearrange("p (c m) -> p c m", m=128))
            for c in range(8):
                eng = nc.sync if c % 2 == 0 else nc.gpsimd
                eng.dma_start(xt_sb[:, c : c + 1, :], xt_r[:, c : c + 1, :])
            nc.gpsimd.dma_start(wv_sb[:], wv.rearrange("p (c m) -> p c m", m=128))
            nc.gpsimd.dma_start(cos_sb[:], cos_d)
            nc.gpsimd.dma_start(sin_sb[:], sin_d)
            nc.scalar.dma_start(bq_sb[:], bq)
            nc.scalar.dma_start(bk_sb[:], bk)
            nc.scalar.dma_start(bvb[:], bv.to_broadcast((128, 128)))
            nc.scalar.dma_start(wo_sb[:], wo.rearrange("p (j m) -> p j m", m=128))

            # ---------------- Phase 1: QKV projections + RoPE -------------
            # ch-outer chains, batch-0 st-halves first: chain (st) for ch
            # needs only xt chunk ch, so the PE tracks the DMA; b0's Q/K
            # finish ~24us and rope(b0) overlaps the b1 half's matmuls.
            with tc.tile_pool(name="qkv_ps", bufs=1, space="PSUM") as qkv_ps:

                def qk_half(w_sb, b_sb, plain, bank0, half):
                    sts = list(range(half * 4, half * 4 + 4))
                    ps = {
                        st: qkv_ps.tile(
                            [128, 512], fp32, tag=f"b{bank0 + i}", name=f"ps{st}"
                        )
                        for i, st in enumerate(sts)
                    }
                    for ch in range(8):
                        for st in sts:
                            nc.tensor.matmul(
                                ps[st][:],
                                w_sb[:, ch, :],
                                xt_sb[:, ch, st * 512 : (st + 1) * 512],
                                start=(ch == 0),
                                stop=(ch == 7),
                            )
                    for st in sts:
                        nc.vector.tensor_scalar_add(
                            plain[:, (st - half * 4) * 512 :][:, 0:512],
                            ps[st][:],
                            b_sb[:],
                        )

                def rope(plain, rot_half):
                    # rot = plain*cos + swap(plain)*sin, sin-mul in place
                    swap = qkv_tmp.tile([128, S], bf16, tag="swap", name="swap")
                    for g in (0, 64):
                        nc.scalar.dma_start(
                            swap[g : g + 32, :], plain[g + 32 : g + 64, :]
                        )
                        nc.scalar.dma_start(
                            swap[g + 32 : g + 64, :], plain[g : g + 32, :]
                        )
                    nc.vector.tensor_mul(rot_half, plain[:], cos_sb[:])
                    nc.vector.tensor_mul(swap[:], swap[:], sin_sb[:])
                    nc.vector.tensor_add(rot_half, rot_half, swap[:])

                for half in range(2):  # batch halves
                    for w_sb, b_sb, key, bank0 in (
                        (wq_sb, bq_sb, "q", 0),
                        (wk_sb, bk_sb, "k", 4),
                    ):
                        plain = qkv_tmp.tile(
                            [128, S], bf16, tag="plain", name=f"plain_{key}{half}"
                        )
                        qk_half(w_sb, b_sb, plain, bank0, half)
                        rot = qrot if key == "q" else krot
                        rope(plain, rot[:, half * S : (half + 1) * S])

                ones_ap = v_sb[:].rearrange("p t (h x) -> p t h x", x=DK + 1)[
                    :, :, :, DK
                ]
                nc.vector.memset(ones_ap, 1.0)

            # ---------------- Phase 2: attention ------------------------
            with (
                tc.tile_pool(name="sc_ps", bufs=2, space="PSUM") as sc_ps,
                tc.tile_pool(name="ctx_ps", bufs=2, space="PSUM") as ctx_ps,
                tc.tile_pool(name="op_ps", bufs=2, space="PSUM") as op_ps,
            ):
                work = deque()

                def pump(n):
                    for _ in range(n):
                        if work:
                            work.popleft()()

                def v_chain(tt):
                    def unit():
                        psv = op_ps.tile(
                            [128, 128], fp32, tag="op", name=f"psv{tt}"
                        )
                        for ch in range(8):
                            nc.tensor.matmul(
                                psv[:],
                                xt_sb[:, ch, tt * 128 : (tt + 1) * 128],
                                wv_sb[:, ch, :],
                                start=(ch == 0),
                                stop=(ch == 7),
                            )
                        dst = v_sb[:, tt].rearrange("p (h x) -> p h x", h=2)[
                            :, :, 0:DK
                        ]
                        nc.vector.tensor_add(dst, psv[:], bvb[:])

                    return unit

                def op_unit(st, oc, copy_eng):
                    def unit():
                        po = op_ps.tile(
                            [128, 512], fp32, tag="op", name=f"po{st}_{oc}"
                        )
                        nc.tensor.matmul(
                            po[:],
                            wo_sb[:, oc, :],
                            ctxT[:, st * 512 : (st + 1) * 512],
                            start=True,
                            stop=True,
                        )
                        ob = op_sb.tile(
                            [128, 512], bf16, tag="ob", name=f"ob{st}_{oc}"
                        )
                        copy_eng(ob[:], po[:])
                        nc.sync.dma_start(
                            out_d[
                                oc * 128 : (oc + 1) * 128,
                                st * 512 : (st + 1) * 512,
                            ],
                            ob[:],
                        )

                    return unit

                def div_unit(b, h):
                    # den row -> DRAM -> [128,16] -> recip -> DRAM ->
                    # partition-broadcast R -> scale ctxT rows
                    def unit():
                        pi = b * HPC + h
                        dr = 32 * pi
                        hh = h * DK
                        nc.gpsimd.dma_start(
                            den_dram[pi * S : (pi + 1) * S].rearrange(
                                "(o c) -> o c", o=1
                            ),
                            den_big[dr : dr + 1, :],
                        )
                        nc.gpsimd.dma_start(
                            dn128[:, pi, :],
                            den_dram[pi * S : (pi + 1) * S].rearrange(
                                "(p c) -> p c", p=128
                            ),
                        )
                        with nc.allow_low_precision(
                            reason="bf16 softmax reciprocal within tolerance"
                        ):
                            nc.vector.reciprocal(rc128[:, pi, :], dn128[:, pi, :])
                        nc.gpsimd.dma_start(
                            rec_dram[pi * S : (pi + 1) * S].rearrange(
                                "(p c) -> p c", p=128
                            ),
                            rc128[:, pi, :],
                        )
                        R = att_sb.tile(
                            [128, S], bf16, tag="R", bufs=1, name=f"R{pi}"
                        )
                        nc.gpsimd.dma_start(
                            R[hh : hh + DK, :],
                            rec_dram[pi * S : (pi + 1) * S]
                            .rearrange("(o s) -> o s", o=1)
                            .to_broadcast((DK, S)),
                        )
                        sl = slice(b * S, (b + 1) * S)
                        nc.vector.tensor_mul(
                            ctxT[hh : hh + DK, sl],
                            ctxT[hh : hh + DK, sl],
                            R[hh : hh + DK, :],
                        )

                    return unit

                def ctx_units(b, sh, si, expA, expB):
                    # two 16-matmul ctx chains (head 0 / head 1) for the
                    # 512-col stripe st_i, as 8 pump units of 4 MMs + one
                    # evacuation unit. Chains ride the NEXT block's exp.
                    st_i = sh * 2 + si
                    pcs = {}

                    def chain_quarter(h, q):
                        def unit():
                            if q == 0:
                                pcs[h] = ctx_ps.tile(
                                    [DK + 1, 512],
                                    fp32,
                                    tag="pc",
                                    name=f"pc{b}{st_i}{h}",
                                )
                            exp = expA if h == 0 else expB
                            for tt in range(q * 4, q * 4 + 4):
                                nc.tensor.matmul(
                                    pcs[h][:],
                                    v_sb[
                                        :,
                                        b * 16 + tt,
                                        h * (DK + 1) : (h + 1) * (DK + 1),
                                    ],
                                    exp[:, tt, :],
                                    start=(tt == 0),
                                    stop=(tt == 15),
                                )

                        return unit

                    def evacuate():
                        ds0 = b * S + st_i * 512
                        for h in range(HPC):
                            pc = pcs[h]
                            pi = b * HPC + h
                            if h == 0:
                                nc.vector.tensor_copy(
                                    ctxT[0:DK, ds0 : ds0 + 512], pc[0:DK, :]
                                )
                            else:
                                stg = att_sb.tile(
                                    [DK, 512], bf16, tag="stg", bufs=1, name="stg"
                                )
                                nc.vector.tensor_copy(stg[:], pc[0:DK, :])
                                nc.vector.stream_shuffle(
                                    ctxT[DK : 2 * DK, ds0 : ds0 + 512],
                                    stg[:],
                                    mask=list(range(32)),
                                )
                            nc.vector.tensor_copy(
                                den_big[
                                    32 * pi : 32 * pi + 1,
                                    st_i * 512 : (st_i + 1) * 512,
                                ],
                                pc[DK : DK + 1, :],
                            )

                    units = []
                    for q in range(4):
                        units.append(chain_quarter(0, q))
                        units.append(chain_quarter(1, q))
                    units.append(evacuate)
                    return units

                def attn_block(b, sh, si, budget):
                    # scores both heads packed (concurrent row-group tiles
                    # (0,0) and (64,0)), exp FD=512, pump in exp shadow
                    expA = att_sb.tile(
                        [128, 16, 512], bf16, tag="expA", bufs=2,
                        name=f"eA{b}{sh}{si}",
                    )
                    expB = att_sb.tile(
                        [128, 16, 512], bf16, tag="expB", bufs=2,
                        name=f"eB{b}{sh}{si}",
                    )
                    s0 = b * S + sh * 1024 + si * 512
                    for tt in range(16):
                        tb = slice(b * S + tt * 128, b * S + (tt + 1) * 128)
                        psA = sc_ps.tile([128, 512], fp32, tag="scA", name="psA")
                        psB = sc_ps.tile([128, 512], fp32, tag="scB", name="psB")
                        nc.tensor.matmul(
                            psA[:], krot[0:DK, tb], qrot[0:DK, s0 : s0 + 512],
                            start=True, stop=True,
                        )
                        nc.tensor.matmul(
                            psB[:], krot[DK:128, tb], qrot[DK:128, s0 : s0 + 512],
                            start=True, stop=True,
                        )
                        nc.scalar.activation(
                            expA[:, tt, :], psA[:], Exp, scale=0.125
                        )
                        nc.scalar.activation(
                            expB[:, tt, :], psB[:], Exp, scale=0.125
                        )
                        pump(budget)
                    return expA, expB

                # V chains become available once all xt chunks landed; the
                # first block's exp shadow absorbs them.
                for tt in range(16):
                    work.append(v_chain(tt))

                blocks = [
                    (b, sh, si)
                    for b in range(B)
                    for sh in range(2)
                    for si in range(2)
                ]
                prev = None
                for bi, (b, sh, si) in enumerate(blocks):
                    if bi == 1:
                        for tt in range(16, 32):
                            work.append(v_chain(tt))
                    if bi == 5:
                        work.append(div_unit(0, 0))
                        work.append(div_unit(0, 1))
                        for st in range(4):
                            for oc in range(8):
                                work.append(
                                    op_unit(st, oc, nc.vector.tensor_copy)
                                )
                    if prev is not None:
                        work.extendleft(reversed(ctx_units(*prev)))
                    expA, expB = attn_block(b, sh, si, budget=2)
                    prev = (b, sh, si, expA, expB)

                # drain: last block's ctx, remaining pump work, b1 divs,
                # b1 out-projection
                for u in ctx_units(*prev):
                    u()
                while work:
                    work.popleft()()
                div_unit(1, 0)()
                div_unit(1, 1)()
                for st in range(4, 8):
                    for oc in range(8):
                        copy_eng = (
                            nc.vector.tensor_copy
                            if (st * 8 + oc) % 2 == 0
                            else nc.scalar.copy
                        )
                        op_unit(st, oc, copy_eng)()

    nc.compile()
    return nc


def _rope_tables():
    pos = np.arange(S, dtype=np.float64)
    inv_freq = np.exp(np.arange(0, DK, 2, dtype=np.float64) * (-np.log(10000.0) / DK))
    ang = pos[:, None] * inv_freq[None, :]  # [S, 32]
    cos_t = np.empty((128, S), dtype=np.float32)
    sin_t = np.empty((128, S), dtype=np.float32)
    c = np.cos(ang).astype(np.float32).T  # [32, S]
    s = np.sin(ang).astype(np.float32).T
    for blk in range(4):
        cos_t[blk * 32 : (blk + 1) * 32] = c
        sign = -1.0 if blk % 2 == 0 else 1.0
        sin_t[blk * 32 : (blk + 1) * 32] = sign * s
    return cos_t, sin_t


def _prep_w(w):
    # [1024, 128] column slice -> [128, 8*128] with the 1024-dim split into
    # 8 chunks of 128 on the partition axis (contiguous 2KB DMA lines)
    bf = ml_dtypes.bfloat16
    return np.ascontiguousarray(
        np.asarray(w, dtype=np.float32)
        .reshape(8, 128, 128)
        .transpose(1, 0, 2)
        .reshape(128, 8 * 128)
    ).astype(bf)


def _prep_inputs(inputs, Wq, bq, Wk, bk, Wv, bv, Wo):
    bf = ml_dtypes.bfloat16
    x2 = np.asarray(inputs, dtype=np.float32).reshape(NS, D)
    xt = np.ascontiguousarray(x2.T).astype(bf)
    cos_t, sin_t = _rope_tables()
    cos_b = cos_t.astype(bf)
    sin_b = sin_t.astype(bf)
    in_maps = []
    for c in range(NCORES):
        sl = slice(c * DPC, (c + 1) * DPC)
        in_maps.append(
            {
                "xt": xt,
                "wq": _prep_w(Wq[:, sl]),
                "wk": _prep_w(Wk[:, sl]),
                "wv": _prep_w(Wv[:, sl]),
                "wo": np.ascontiguousarray(Wo[sl, :]).astype(bf),
                "bq": np.ascontiguousarray(bq[sl]).reshape(DPC, 1).astype(np.float32),
                "bk": np.ascontiguousarray(bk[sl]).reshape(DPC, 1).astype(np.float32),
                "bv": np.ascontiguousarray(bv[sl]).reshape(1, DPC).astype(bf),
                "cos": cos_b,
                "sin": sin_b,
            }
        )
    return in_maps


def _get_nc():
    if "nc" not in _cache:
        _cache["nc"] = _build_nc()
    return _cache["nc"]


def run(inputs_dict, trace=False):
    """Build (cached), run on 8 cores, assemble full output. Returns
    (output fp32 [B,S,D], BassKernelResults)."""
    from concourse.bass_utils import run_bass_kernel_spmd

    nc = _get_nc()
    in_maps = _prep_inputs(
        inputs_dict["inputs"],
        inputs_dict["Wq"],
        inputs_dict["bq"],
        inputs_dict["Wk"],
        inputs_dict["bk"],
        inputs_dict["Wv"],
        inputs_dict["bv"],
        inputs_dict["Wo"],
    )
    res = run_bass_kernel_spmd(
        nc, in_maps, core_ids=list(range(NCORES)), trace=trace
    )
    acc = np.zeros((D, NS), dtype=np.float32)
    for r in res.results:
        acc += r["out"].astype(np.float32)
    out = acc.T.reshape(B, S, D) + np.asarray(inputs_dict["bo"], dtype=np.float32)
    return out.astype(np.float32), res


def kernel(**inputs):
    out, _ = run(inputs, trace=False)
    return out


# revision 15
# speedup vs baseline: 1.1572x; 1.0469x over previous
"""Trainium2 Bass kernel: 16-head attention with RoPE (dense_transformer).

Sharding: tensor-parallel over heads. 8 cores x 2 heads each.
Each core: Wq/Wk/Wv column slice [1024,128], Wo row slice [128,1024],
full input; computes its heads' attention + partial output projection.
Host sums the 8 partial outputs (row-parallel Wo reduction) and adds bo.

Device layout is "transposed": Q^T/K^T/ctx^T are kept as [dim, seq] with
the head dim on SBUF partitions, so Q^T = Wq^T @ X^T comes straight out
of the PE, scores^T = K^T.T @ Q^T needs no transposes, and the softmax
denominator falls out of an extra ones-column appended to V.

v4 schedule: the kernel is paced by the softmax exp (only the ACT
engine has an exp LUT; ACTIVATE costs ~(FD+313)ns so 16.8M elements
cost 142us at FD=1024). Everything else hides in its shadow:
 - xt arrives as 8 channel-major chunks (8KB DMA lines); batch-0's Q/K
   chains interleave per chunk, so rope(b0) starts ~26us and the first
   exp fires ~34us. Batch-1's Q/K chains + rope run as pump units
   inside the first attention blocks' exp shadow.
 - the two heads' score matmuls issue back-to-back as concurrent PE
   row-group tiles (0,0)/(64,0) into the two banks of ONE [128,1024]
   PSUM tile, so a single FD=1024 exp covers both heads.
 - each block's ctx chains ride the NEXT block's exp shadow, as do the
   V-projection chains and batch-0's output projection ("pump" units).
 - softmax division: reciprocal on the den row, gpsimd
   partition_broadcast (no DRAM bounce), one multiply per head.
 - output tiles leave via [128,4,512] quad-DMAs on alternating queues
   (per-tile DMAs pay ~2us completion latency each).
"""

import sys

if "/opt/trn_rl_repo" not in sys.path:
    sys.path.insert(0, "/opt/trn_rl_repo")

from collections import deque

import numpy as np
import ml_dtypes

B = 2
S = 2048
NS = B * S  # 4096
D = 1024
H = 16
DK = 64
NCORES = 8
HPC = H // NCORES  # heads per core = 2
DPC = HPC * DK  # model dims per core = 128

_cache = {}


def _build_nc():
    import concourse.bass as bass
    import concourse.tile as tile
    import concourse.mybir as mybir
    from concourse import bacc

    fp32 = mybir.dt.float32
    bf16 = mybir.dt.bfloat16
    Exp = mybir.ActivationFunctionType.Exp

    nc = bacc.Bacc("TRN2", debug=False, num_devices=NCORES)

    xt = nc.dram_tensor("xt", [D, NS], bf16, kind="ExternalInput").ap()
    wq = nc.dram_tensor("wq", [128, 8 * 128], bf16, kind="ExternalInput").ap()
    wk = nc.dram_tensor("wk", [128, 8 * 128], bf16, kind="ExternalInput").ap()
    wv = nc.dram_tensor("wv", [128, 8 * 128], bf16, kind="ExternalInput").ap()
    wo = nc.dram_tensor("wo", [DPC, D], bf16, kind="ExternalInput").ap()
    bq = nc.dram_tensor("bq", [DPC, 1], fp32, kind="ExternalInput").ap()
    bk = nc.dram_tensor("bk", [DPC, 1], fp32, kind="ExternalInput").ap()
    bv = nc.dram_tensor("bv", [1, DPC], bf16, kind="ExternalInput").ap()
    cos_d = nc.dram_tensor("cos", [128, S], bf16, kind="ExternalInput").ap()
    sin_d = nc.dram_tensor("sin", [128, S], bf16, kind="ExternalInput").ap()
    out_d = nc.dram_tensor("out", [D, NS], bf16, kind="ExternalOutput").ap()

    with tile.TileContext(nc) as tc:
        with (
            tc.tile_pool(name="persist", bufs=1) as persist,
            tc.tile_pool(name="qkv_sb", bufs=1) as qkv_sb,
            tc.tile_pool(name="qkv_tmp", bufs=1) as qkv_tmp,
            tc.tile_pool(name="att_sb", bufs=1) as att_sb,
            tc.tile_pool(name="op_sb", bufs=2) as op_sb,
            tc.tile_pool(name="dram", bufs=1, space="DRAM") as dram,
        ):
            qrot = persist.tile([128, NS], bf16, tag="qrot")
            krot = persist.tile([128, NS], bf16, tag="krot")
            # v_sb[:, tt, 65h : 65h+64] = V rows tt*128.. for head h,
            # v_sb[:, tt, 65h+64] = 1.0 (denominator column)
            v_sb = persist.tile([128, 32, 2 * (DK + 1)], bf16, tag="v")
            ctxT = persist.tile([128, NS], bf16, tag="ctxT")
            wo_sb = persist.tile([128, 8, 128], bf16, tag="wo")
            # den chunks land on 32-aligned rows: row 32*(b*2+h), cols
            # st_i*512.. (TensorCopy across partitions needs 32-aligned
            # bases; tensor_tensor ops need exact matches)
            den_big = persist.tile([128, S], bf16, tag="den_big")
            rec_big = persist.tile([128, S], bf16, tag="rec_big")
            rec_dram = dram.tile([4 * S], bf16, tag="rec_dram")

            cos_sb = qkv_sb.tile([128, S], bf16, tag="cos")
            sin_sb = qkv_sb.tile([128, S], bf16, tag="sin")
            wq_sb = qkv_sb.tile([128, 8, 128], bf16, tag="wq")
            wk_sb = qkv_sb.tile([128, 8, 128], bf16, tag="wk")
            wv_sb = qkv_sb.tile([128, 8, 128], bf16, tag="wv")
            bq_sb = qkv_sb.tile([128, 1], fp32, tag="bq")
            bk_sb = qkv_sb.tile([128, 1], fp32, tag="bk")
            bvb = qkv_sb.tile([128, 128], bf16, tag="bvb")
            xt_sb = qkv_sb.tile([128, 8, NS], bf16, tag="xt")
            xt_r = xt.rearrange("(c p) s -> p c s", p=128)
            out_r = out_d.rearrange("(j p) s -> p j s", p=128)

            # DMA plan: 3 queues. sync: wq + even chunks; gpsimd: wk + odd
            # chunks + wv/cos/sin; scalar(ACT, idle now): small tail.
            nc.sync.dma_start(wq_sb[:], wq.rearrange("p (c m) -> p c m", m=128))
            nc.gpsimd.dma_start(wk_sb[:], wk.rearrange("p (c m) -> p c m", m=128))
            for c in range(8):
                eng = nc.sync if c % 2 == 0 else nc.gpsimd
                eng.dma_start(xt_sb[:, c : c + 1, :], xt_r[:, c : c + 1, :])
            nc.gpsimd.dma_start(wv_sb[:], wv.rearrange("p (c m) -> p c m", m=128))
            nc.gpsimd.dma_start(cos_sb[:], cos_d)
            nc.gpsimd.dma_start(sin_sb[:], sin_d)
            nc.scalar.dma_start(bq_sb[:], bq)
            nc.scalar.dma_start(bk_sb[:], bk)
            nc.scalar.dma_start(bvb[:], bv.to_broadcast((128, 128)))
            nc.scalar.dma_start(wo_sb[:], wo.rearrange("p (j m) -> p j m", m=128))

            def rope(plain, rot_half):
                # rot = plain*cos + swap(plain)*sin, sin-mul in place
                swap = qkv_tmp.tile([128, S], bf16, tag="swap", name="swap")
                for g in (0, 64):
                    nc.scalar.dma_start(
                        swap[g : g + 32, :], plain[g + 32 : g + 64, :]
                    )
                    nc.scalar.dma_start(
                        swap[g + 32 : g + 64, :], plain[g : g + 32, :]
                    )
                nc.vector.tensor_mul(rot_half, plain[:], cos_sb[:])
                nc.vector.tensor_mul(swap[:], swap[:], sin_sb[:])
                nc.vector.tensor_add(rot_half, rot_half, swap[:])

            # -------- Phase 1: batch-0 Q/K projections + RoPE -------------
            # Q and K chains interleave per xt chunk so both track the DMA;
            # done ~26us in. The batch-1 halves run later as pump units.
            plain_q0 = qkv_tmp.tile([128, S], bf16, tag="plain", name="plain_q0")
            plain_k0 = qkv_tmp.tile([128, S], bf16, tag="plain2", name="plain_k0")
            with tc.tile_pool(name="qkv_ps", bufs=1, space="PSUM") as qkv_ps:
                ps_q = [
                    qkv_ps.tile([128, 512], fp32, tag=f"b{i}", name=f"psq{i}")
                    for i in range(4)
                ]
                ps_k = [
                    qkv_ps.tile([128, 512], fp32, tag=f"b{4 + i}", name=f"psk{i}")
                    for i in range(4)
                ]
                for ch in range(8):
                    for st in range(4):
                        nc.tensor.matmul(
                            ps_q[st][:],
                            wq_sb[:, ch, :],
                            xt_sb[:, ch, st * 512 : (st + 1) * 512],
                            start=(ch == 0),
                            stop=(ch == 7),
                        )
                    for st in range(4):
                        nc.tensor.matmul(
                            ps_k[st][:],
                            wk_sb[:, ch, :],
                            xt_sb[:, ch, st * 512 : (st + 1) * 512],
                            start=(ch == 0),
                            stop=(ch == 7),
                        )
                for st in range(4):
                    nc.vector.tensor_scalar_add(
                        plain_q0[:, st * 512 : (st + 1) * 512], ps_q[st][:], bq_sb[:]
                    )
                for st in range(4):
                    nc.vector.tensor_scalar_add(
                        plain_k0[:, st * 512 : (st + 1) * 512], ps_k[st][:], bk_sb[:]
                    )
                rope(plain_q0, qrot[:, 0:S])
                rope(plain_k0, krot[:, 0:S])

                ones_ap = v_sb[:].rearrange("p t (h x) -> p t h x", x=DK + 1)[
                    :, :, :, DK
                ]
                nc.vector.memset(ones_ap, 1.0)

            # ---------------- Phase 2: attention ------------------------
            with (
                tc.tile_pool(name="sc_ps", bufs=2, space="PSUM") as sc_ps,
                tc.tile_pool(name="ctx_ps", bufs=2, space="PSUM") as ctx_ps,
                tc.tile_pool(name="op_ps", bufs=2, space="PSUM") as op_ps,
            ):
                work = deque()

                def pump(n):
                    for _ in range(n):
                        if work:
                            work.popleft()()

                def v_chain(tt):
                    def unit():
                        psv = op_ps.tile(
                            [128, 128], fp32, tag="op", name=f"psv{tt}"
                        )
                        for ch in range(8):
                            nc.tensor.matmul(
                                psv[:],
                                xt_sb[:, ch, tt * 128 : (tt + 1) * 128],
                                wv_sb[:, ch, :],
                                start=(ch == 0),
                                stop=(ch == 7),
                            )
                        dst = v_sb[:, tt].rearrange("p (h x) -> p h x", h=2)[
                            :, :, 0:DK
                        ]
                        nc.vector.tensor_add(dst, psv[:], bvb[:])

                    return unit

                def qk_b1_chain(w_sb, b_sb, plain, st):
                    def unit():
                        psq = op_ps.tile(
                            [128, 512], fp32, tag="op", name=f"qk1_{st}"
                        )
                        for ch in range(8):
                            nc.tensor.matmul(
                                psq[:],
                                w_sb[:, ch, :],
                                xt_sb[:, ch, st * 512 : (st + 1) * 512],
                                start=(ch == 0),
                                stop=(ch == 7),
                            )
                        nc.vector.tensor_scalar_add(
                            plain[:, (st - 4) * 512 :][:, 0:512], psq[:], b_sb[:]
                        )

                    return unit

                def op_quad(st, j, engs):
                    # 2 out-proj tiles (oc = 2j, 2j+1) -> one 128KB DMA;
                    # per-tile DMAs pay ~2us completion latency each
                    def unit():
                        ob = op_sb.tile(
                            [128, 2, 512], bf16, tag="ob", name=f"ob{st}_{j}"
                        )
                        for k in range(2):
                            oc = j * 2 + k
                            po = op_ps.tile(
                                [128, 512], fp32, tag="op", name=f"po{st}_{oc}"
                            )
                            nc.tensor.matmul(
                                po[:],
                                wo_sb[:, oc, :],
                                ctxT[:, st * 512 : (st + 1) * 512],
                                start=True,
                                stop=True,
                            )
                            engs[k % len(engs)](ob[:, k, :], po[:])
                        dq = nc.sync if (st + j) % 2 == 0 else nc.gpsimd
                        dq.dma_start(
                            out_r[:, j * 2 : j * 2 + 2, st * 512 : (st + 1) * 512],
                            ob[:],
                        )

                    return unit

                def div_unit(b, h, dq):
                    # reciprocal of the den row, DRAM bounce to broadcast
                    # along partitions, scale this head's ctxT rows
                    def unit():
                        pi = b * HPC + h
                        dr = 32 * pi
                        hh = h * DK
                        with nc.allow_low_precision(
                            reason="bf16 softmax reciprocal within tolerance"
                        ):
                            nc.vector.reciprocal(
                                rec_big[dr : dr + 1, :], den_big[dr : dr + 1, :]
                            )
                        nc.gpsimd.dma_start(
                            rec_dram[pi * S : (pi + 1) * S].rearrange(
                                "(o c) -> o c", o=1
                            ),
                            rec_big[dr : dr + 1, :],
                        )
                        R = att_sb.tile(
                            [128, S], bf16, tag="R", bufs=1, name=f"R{pi}"
                        )
                        dq.dma_start(
                            R[hh : hh + DK, :],
                            rec_dram[pi * S : (pi + 1) * S]
                            .rearrange("(o s) -> o s", o=1)
                            .to_broadcast((DK, S)),
                        )
                        sl = slice(b * S, (b + 1) * S)
                        nc.vector.tensor_mul(
                            ctxT[hh : hh + DK, sl],
                            ctxT[hh : hh + DK, sl],
                            R[hh : hh + DK, :],
                        )

                    return unit

                def ctx_units(b, sh, si, expS):
                    # two 16-matmul ctx chains (head 0 / head 1) for the
                    # 512-col stripe st_i, as 8 pump units of 4 MMs + one
                    # evacuation unit. Chains ride the NEXT block's exp.
                    st_i = sh * 2 + si
                    pcs = {}

                    def chain_quarter(h, q):
                        def unit():
                            if q == 0:
                                pcs[h] = ctx_ps.tile(
                                    [DK + 1, 512],
                                    fp32,
                                    tag="pc",
                                    name=f"pc{b}{st_i}{h}",
                                )
                            for tt in range(q * 4, q * 4 + 4):
                                nc.tensor.matmul(
                                    pcs[h][:],
                                    v_sb[
                                        :,
                                        b * 16 + tt,
                                        h * (DK + 1) : (h + 1) * (DK + 1),
                                    ],
                                    expS[:, tt, h * 512 : (h + 1) * 512],
                                    start=(tt == 0),
                                    stop=(tt == 15),
                                )

                        return unit

                    def evacuate():
                        ds0 = b * S + st_i * 512
                        for h in range(HPC):
                            pc = pcs[h]
                            pi = b * HPC + h
                            if h == 0:
                                nc.vector.tensor_copy(
                                    ctxT[0:DK, ds0 : ds0 + 512], pc[0:DK, :]
                                )
                            else:
                                stg = att_sb.tile(
                                    [DK, 512], bf16, tag="stg", bufs=1, name="stg"
                                )
                                nc.vector.tensor_copy(stg[:], pc[0:DK, :])
                                nc.vector.stream_shuffle(
                                    ctxT[DK : 2 * DK, ds0 : ds0 + 512],
                                    stg[:],
                                    mask=list(range(32)),
                                )
                            nc.vector.tensor_copy(
                                den_big[
                                    32 * pi : 32 * pi + 1,
                                    st_i * 512 : (st_i + 1) * 512,
                                ],
                                pc[DK : DK + 1, :],
                            )

                    units = []
                    for q in range(4):
                        units.append(chain_quarter(0, q))
                        units.append(chain_quarter(1, q))
                    units.append(evacuate)
                    return units

                def attn_block(b, sh, si, budget):
                    # both heads' scores into the two banks of one
                    # [128,1024] PSUM tile (concurrent row-group tiles
                    # (0,0)/(64,0)); ONE FD=1024 exp covers both heads.
                    # expS[:, tt, 0:512]=head0, [:, tt, 512:]=head1.
                    expS = att_sb.tile(
                        [128, 16, 1024], bf16, tag="expS", bufs=2,
                        name=f"eS{b}{sh}{si}",
                    )
                    s0 = b * S + sh * 1024 + si * 512
                    for tt in range(16):
                        pump(budget)
                        tb = slice(b * S + tt * 128, b * S + (tt + 1) * 128)
                        ps = sc_ps.tile([128, 1024], fp32, tag="sc", name="psAB")
                        nc.tensor.matmul(
                            ps[:, 0:512], krot[0:DK, tb], qrot[0:DK, s0 : s0 + 512],
                            start=True, stop=True,
                        )
                        nc.tensor.matmul(
                            ps[:, 512:1024],
                            krot[DK:128, tb],
                            qrot[DK:128, s0 : s0 + 512],
                            start=True, stop=True,
                        )
                        nc.scalar.activation(expS[:, tt, :], ps[:], Exp, scale=0.125)
                    return expS

                # pump inventory: V(b0) chains first (xt fully landed by the
                # first block), then batch-1 Q/K chains + their rope, then
                # V(b1); ctx units always jump the queue (extendleft).
                for tt in range(16):
                    work.append(v_chain(tt))
                plain_q1 = qkv_tmp.tile(
                    [128, S], bf16, tag="plain", name="plain_q1"
                )
                plain_k1 = qkv_tmp.tile(
                    [128, S], bf16, tag="plain2", name="plain_k1"
                )
                for st in range(4, 8):
                    work.append(qk_b1_chain(wq_sb, bq_sb, plain_q1, st))
                work.append(lambda: rope(plain_q1, qrot[:, S:NS]))
                for st in range(4, 8):
                    work.append(qk_b1_chain(wk_sb, bk_sb, plain_k1, st))
                work.append(lambda: rope(plain_k1, krot[:, S:NS]))

                blocks = [
                    (b, sh, si)
                    for b in range(B)
                    for sh in range(2)
                    for si in range(2)
                ]
                prev = None
                for bi, (b, sh, si) in enumerate(blocks):
                    if bi == 1:
                        for tt in range(16, 32):
                            work.append(v_chain(tt))
                    if bi == 5:
                        work.append(div_unit(0, 0, nc.gpsimd))
                        work.append(div_unit(0, 1, nc.gpsimd))
                        engs = [nc.vector.tensor_copy]
                        for st in range(4):
                            for j in range(4):
                                work.append(op_quad(st, j, engs))
                    if prev is not None:
                        work.extendleft(reversed(ctx_units(*prev)))
                    expS = attn_block(b, sh, si, budget=2)
                    prev = (b, sh, si, expS)

                # drain: last block's ctx, remaining pump work, b1 divs,
                # b1 out-projection
                for u in ctx_units(*prev):
                    u()
                while work:
                    work.popleft()()
                div_unit(1, 0, nc.gpsimd)()
                div_unit(1, 1, nc.sync)()
                engs = [nc.vector.tensor_copy, nc.scalar.copy]
                for st in range(4, 8):
                    for j in range(4):
                        op_quad(st, j, engs)()

    nc.compile()
    return nc


def _rope_tables():
    pos = np.arange(S, dtype=np.float64)
    inv_freq = np.exp(np.arange(0, DK, 2, dtype=np.float64) * (-np.log(10000.0) / DK))
    ang = pos[:, None] * inv_freq[None, :]  # [S, 32]
    cos_t = np.empty((128, S), dtype=np.float32)
    sin_t = np.empty((128, S), dtype=np.float32)
    c = np.cos(ang).astype(np.float32).T  # [32, S]
    s = np.sin(ang).astype(np.float32).T
    for blk in range(4):
        cos_t[blk * 32 : (blk + 1) * 32] = c
        sign = -1.0 if blk % 2 == 0 else 1.0
        sin_t[blk * 32 : (blk + 1) * 32] = sign * s
    return cos_t, sin_t


def _prep_w(w):
    # [1024, 128] column slice -> [128, 8*128] with the 1024-dim split into
    # 8 chunks of 128 on the partition axis (contiguous 2KB DMA lines)
    bf = ml_dtypes.bfloat16
    return np.ascontiguousarray(
        np.asarray(w, dtype=np.float32)
        .reshape(8, 128, 128)
        .transpose(1, 0, 2)
        .reshape(128, 8 * 128)
    ).astype(bf)


def _prep_inputs(inputs, Wq, bq, Wk, bk, Wv, bv, Wo):
    bf = ml_dtypes.bfloat16
    x2 = np.asarray(inputs, dtype=np.float32).reshape(NS, D)
    xt = np.ascontiguousarray(x2.T).astype(bf)
    cos_t, sin_t = _rope_tables()
    cos_b = cos_t.astype(bf)
    sin_b = sin_t.astype(bf)
    in_maps = []
    for c in range(NCORES):
        sl = slice(c * DPC, (c + 1) * DPC)
        in_maps.append(
            {
                "xt": xt,
                "wq": _prep_w(Wq[:, sl]),
                "wk": _prep_w(Wk[:, sl]),
                "wv": _prep_w(Wv[:, sl]),
                "wo": np.ascontiguousarray(Wo[sl, :]).astype(bf),
                "bq": np.ascontiguousarray(bq[sl]).reshape(DPC, 1).astype(np.float32),
                "bk": np.ascontiguousarray(bk[sl]).reshape(DPC, 1).astype(np.float32),
                "bv": np.ascontiguousarray(bv[sl]).reshape(1, DPC).astype(bf),
                "cos": cos_b,
                "sin": sin_b,
            }
        )
    return in_maps


def _get_nc():
    if "nc" not in _cache:
        _cache["nc"] = _build_nc()
    return _cache["nc"]


def run(inputs_dict, trace=False):
    """Build (cached), run on 8 cores, assemble full output. Returns
    (output fp32 [B,S,D], BassKernelResults)."""
    from concourse.bass_utils import run_bass_kernel_spmd

    nc = _get_nc()
    in_maps = _prep_inputs(
        inputs_dict["inputs"],
        inputs_dict["Wq"],
        inputs_dict["bq"],
        inputs_dict["Wk"],
        inputs_dict["bk"],
        inputs_dict["Wv"],
        inputs_dict["bv"],
        inputs_dict["Wo"],
    )
    res = run_bass_kernel_spmd(
        nc, in_maps, core_ids=list(range(NCORES)), trace=trace
    )
    acc = np.zeros((D, NS), dtype=np.float32)
    for r in res.results:
        acc += r["out"].astype(np.float32)
    out = acc.T.reshape(B, S, D) + np.asarray(inputs_dict["bo"], dtype=np.float32)
    return out.astype(np.float32), res


def kernel(**inputs):
    out, _ = run(inputs, trace=False)
    return out


# revision 17
# speedup vs baseline: 1.3930x; 1.2038x over previous
"""Trainium2 Bass kernel: 16-head attention with RoPE (dense_transformer).

Sharding: tensor-parallel over heads. 8 cores x 2 heads each.
Each core: Wq/Wk/Wv column slice [1024,128], Wo row slice [128,1024],
full input; computes its heads' attention + partial output projection.
Host sums the 8 partial outputs (row-parallel Wo reduction) and adds bo.

Device layout is "transposed": Q^T/K^T/ctx^T are kept as [dim, seq] with
the head dim on SBUF partitions, so Q^T = Wq^T @ X^T comes straight out
of the PE, scores^T = K^T.T @ Q^T needs no transposes, and the softmax
denominator falls out of an extra ones-column appended to V.

v4 schedule: the kernel is paced by the softmax exp (only the ACT
engine has an exp LUT; ACTIVATE costs ~(FD+313)ns so 16.8M elements
cost 142us at FD=1024). Everything else hides in its shadow:
 - xt arrives as 8 channel-major chunks (8KB DMA lines); batch-0's Q/K
   chains interleave per chunk, so rope(b0) starts ~26us and the first
   exp fires ~34us. Batch-1's Q/K chains + rope run as pump units
   inside the first attention blocks' exp shadow.
 - the two heads' score matmuls issue back-to-back as concurrent PE
   row-group tiles (0,0)/(64,0) into the two banks of ONE [128,1024]
   PSUM tile, so a single FD=1024 exp covers both heads.
 - each block's ctx chains ride the NEXT block's exp shadow, as do the
   V-projection chains and batch-0's output projection ("pump" units).
 - softmax division: reciprocal on the den row, gpsimd
   partition_broadcast (no DRAM bounce), one multiply per head.
 - output tiles leave via [128,4,512] quad-DMAs on alternating queues
   (per-tile DMAs pay ~2us completion latency each).
"""

import sys

if "/opt/trn_rl_repo" not in sys.path:
    sys.path.insert(0, "/opt/trn_rl_repo")

from collections import deque

import numpy as np
import ml_dtypes

B = 2
S = 2048
NS = B * S  # 4096
D = 1024
H = 16
DK = 64
NCORES = 8
HPC = H // NCORES  # heads per core = 2
DPC = HPC * DK  # model dims per core = 128

_cache = {}


def _build_nc():
    import concourse.bass as bass
    import concourse.tile as tile
    import concourse.mybir as mybir
    from concourse import bacc

    fp32 = mybir.dt.float32
    bf16 = mybir.dt.bfloat16
    Exp = mybir.ActivationFunctionType.Exp

    nc = bacc.Bacc("TRN2", debug=False, num_devices=NCORES)

    xt = nc.dram_tensor("xt", [D, NS], bf16, kind="ExternalInput").ap()
    wq = nc.dram_tensor("wq", [128, 8 * 128], bf16, kind="ExternalInput").ap()
    wk = nc.dram_tensor("wk", [128, 8 * 128], bf16, kind="ExternalInput").ap()
    wv = nc.dram_tensor("wv", [128, 8 * 128], bf16, kind="ExternalInput").ap()
    wo = nc.dram_tensor("wo", [DPC, D], bf16, kind="ExternalInput").ap()
    bq = nc.dram_tensor("bq", [DPC, 1], fp32, kind="ExternalInput").ap()
    bk = nc.dram_tensor("bk", [DPC, 1], fp32, kind="ExternalInput").ap()
    bv = nc.dram_tensor("bv", [1, DPC], bf16, kind="ExternalInput").ap()
    cos_d = nc.dram_tensor("cos", [128, S], bf16, kind="ExternalInput").ap()
    sin_d = nc.dram_tensor("sin", [128, S], bf16, kind="ExternalInput").ap()
    out_d = nc.dram_tensor("out", [D, NS], bf16, kind="ExternalOutput").ap()

    with tile.TileContext(nc) as tc:
        with (
            tc.tile_pool(name="persist", bufs=1) as persist,
            tc.tile_pool(name="qkv_sb", bufs=1) as qkv_sb,
            tc.tile_pool(name="qkv_tmp", bufs=1) as qkv_tmp,
            tc.tile_pool(name="att_sb", bufs=1) as att_sb,
            tc.tile_pool(name="op_sb", bufs=2) as op_sb,
            tc.tile_pool(name="dram", bufs=1, space="DRAM") as dram,
        ):
            qrot = persist.tile([128, NS], bf16, tag="qrot")
            krot = persist.tile([128, NS], bf16, tag="krot")
            # v_sb[:, tt, 65h : 65h+64] = V rows tt*128.. for head h,
            # v_sb[:, tt, 65h+64] = 1.0 (denominator column)
            v_sb = persist.tile([128, 32, 2 * (DK + 1)], bf16, tag="v")
            ctxT = persist.tile([128, NS], bf16, tag="ctxT")
            wo_sb = persist.tile([128, 8, 128], bf16, tag="wo")
            # den chunks land on 32-aligned rows: row 32*(b*2+h), cols
            # st_i*512.. (TensorCopy across partitions needs 32-aligned
            # bases; tensor_tensor ops need exact matches)
            den_big = persist.tile([128, S], bf16, tag="den_big")
            dn128 = persist.tile([128, 4, 16], bf16, tag="dn128")
            rc128 = persist.tile([128, 4, 16], bf16, tag="rc128")
            den_dram = dram.tile([4 * S], bf16, tag="den_dram")
            rec_dram = dram.tile([4 * S], bf16, tag="rec_dram")

            cos_sb = qkv_sb.tile([128, S], bf16, tag="cos")
            sin_sb = qkv_sb.tile([128, S], bf16, tag="sin")
            wq_sb = qkv_sb.tile([128, 8, 128], bf16, tag="wq")
            wk_sb = qkv_sb.tile([128, 8, 128], bf16, tag="wk")
            wv_sb = qkv_sb.tile([128, 8, 128], bf16, tag="wv")
            bq_sb = qkv_sb.tile([128, 1], fp32, tag="bq")
            bk_sb = qkv_sb.tile([128, 1], fp32, tag="bk")
            bvb = qkv_sb.tile([128, 128], bf16, tag="bvb")
            xt_sb = qkv_sb.tile([128, 8, NS], bf16, tag="xt")
            xt_r = xt.rearrange("(c p) s -> p c s", p=128)
            out_r = out_d.rearrange("(j p) s -> p j s", p=128)

            # DMA plan: 3 queues. sync: wq + even chunks; gpsimd: wk + odd
            # chunks + wv/cos/sin; scalar(ACT, idle now): small tail.
            nc.sync.dma_start(wq_sb[:], wq.rearrange("p (c m) -> p c m", m=128))
            nc.gpsimd.dma_start(wk_sb[:], wk.rearrange("p (c m) -> p c m", m=128))
            for c in range(8):
                eng = nc.sync if c % 2 == 0 else nc.gpsimd
                eng.dma_start(xt_sb[:, c : c + 1, :], xt_r[:, c : c + 1, :])
            nc.scalar.dma_start(bq_sb[:], bq)
            nc.scalar.dma_start(bk_sb[:], bk)
            nc.scalar.dma_start(cos_sb[:], cos_d)
            nc.scalar.dma_start(sin_sb[:], sin_d)
            nc.scalar.dma_start(bvb[:], bv.to_broadcast((128, 128)))
            nc.scalar.dma_start(wv_sb[:], wv.rearrange("p (c m) -> p c m", m=128))
            nc.scalar.dma_start(wo_sb[:], wo.rearrange("p (j m) -> p j m", m=128))

            def rope(plain, rot_half):
                # rot = plain*cos + swap(plain)*sin, sin-mul in place
                swap = qkv_tmp.tile(
                    [128, S], bf16, tag="swap", bufs=2, name="swap"
                )
                for g in (0, 64):
                    nc.scalar.dma_start(
                        swap[g : g + 32, :], plain[g + 32 : g + 64, :]
                    )
                    nc.scalar.dma_start(
                        swap[g + 32 : g + 64, :], plain[g : g + 32, :]
                    )
                nc.vector.tensor_mul(rot_half, plain[:], cos_sb[:])
                nc.vector.tensor_mul(swap[:], swap[:], sin_sb[:])
                nc.vector.tensor_add(rot_half, rot_half, swap[:])

            # -------- Phase 1: batch-0 Q/K projections + RoPE -------------
            # Q and K chains interleave per xt chunk so both track the DMA;
            # done ~26us in. The batch-1 halves run later as pump units.
            plain_q0 = qkv_tmp.tile([128, S], bf16, tag="plain", name="plain_q0")
            plain_k0 = qkv_tmp.tile([128, S], bf16, tag="plain2", name="plain_k0")
            with tc.tile_pool(name="qkv_ps", bufs=1, space="PSUM") as qkv_ps:
                ps_q = [
                    qkv_ps.tile([128, 512], fp32, tag=f"b{i}", name=f"psq{i}")
                    for i in range(4)
                ]
                ps_k = [
                    qkv_ps.tile([128, 512], fp32, tag=f"b{4 + i}", name=f"psk{i}")
                    for i in range(4)
                ]
                for ch in range(8):
                    for st in range(4):
                        nc.tensor.matmul(
                            ps_q[st][:],
                            wq_sb[:, ch, :],
                            xt_sb[:, ch, st * 512 : (st + 1) * 512],
                            start=(ch == 0),
                            stop=(ch == 7),
                        )
                    for st in range(4):
                        nc.tensor.matmul(
                            ps_k[st][:],
                            wk_sb[:, ch, :],
                            xt_sb[:, ch, st * 512 : (st + 1) * 512],
                            start=(ch == 0),
                            stop=(ch == 7),
                        )
                Ident = mybir.ActivationFunctionType.Identity
                for st in range(4):
                    nc.scalar.activation(
                        plain_q0[:, st * 512 : (st + 1) * 512],
                        ps_q[st][:],
                        Ident,
                        bias=bq_sb[:],
                    )
                for st in range(4):
                    nc.scalar.activation(
                        plain_k0[:, st * 512 : (st + 1) * 512],
                        ps_k[st][:],
                        Ident,
                        bias=bk_sb[:],
                    )
                rope(plain_q0, qrot[:, 0:S])
                rope(plain_k0, krot[:, 0:S])

                ones_ap = v_sb[:].rearrange("p t (h x) -> p t h x", x=DK + 1)[
                    :, :, :, DK
                ]
                nc.vector.memset(ones_ap, 1.0)

            # ---------------- Phase 2: attention ------------------------
            with (
                tc.tile_pool(name="sc_ps", bufs=2, space="PSUM") as sc_ps,
                tc.tile_pool(name="ctx_ps", bufs=2, space="PSUM") as ctx_ps,
                tc.tile_pool(name="op_ps", bufs=2, space="PSUM") as op_ps,
            ):
                work = deque()

                def pump(n):
                    for _ in range(n):
                        if work:
                            work.popleft()()

                def v_chain(tt):
                    def unit():
                        psv = op_ps.tile(
                            [128, 128], fp32, tag="op", name=f"psv{tt}"
                        )
                        for ch in range(8):
                            nc.tensor.matmul(
                                psv[:],
                                xt_sb[:, ch, tt * 128 : (tt + 1) * 128],
                                wv_sb[:, ch, :],
                                start=(ch == 0),
                                stop=(ch == 7),
                            )
                        dst = v_sb[:, tt].rearrange("p (h x) -> p h x", h=2)[
                            :, :, 0:DK
                        ]
                        nc.vector.tensor_add(dst, psv[:], bvb[:])

                    return unit

                def qk_b1_chain(w_sb, b_sb, plain, st):
                    def unit():
                        psq = op_ps.tile(
                            [128, 512], fp32, tag="op", name=f"qk1_{st}"
                        )
                        for ch in range(8):
                            nc.tensor.matmul(
                                psq[:],
                                w_sb[:, ch, :],
                                xt_sb[:, ch, st * 512 : (st + 1) * 512],
                                start=(ch == 0),
                                stop=(ch == 7),
                            )
                        nc.vector.tensor_scalar_add(
                            plain[:, (st - 4) * 512 :][:, 0:512], psq[:], b_sb[:]
                        )

                    return unit

                _ob_cycle = [("ob", op_sb), ("plain", qkv_tmp), ("ob", op_sb),
                             ("plain2", qkv_tmp)]

                def op_quad(st, j, engs):
                    # 2 out-proj tiles (oc = 2j, 2j+1) -> one 128KB DMA;
                    # 4-deep staging rotation (op_sb + dead rope slots)
                    # hides the ~2us DMA completion latency
                    def unit():
                        tagname, pool = _ob_cycle[(st * 4 + j) % 4]
                        ob = pool.tile(
                            [128, 2, 512], bf16, tag=tagname, bufs=None,
                            name=f"ob{st}_{j}",
                        )
                        for k in range(2):
                            oc = j * 2 + k
                            po = op_ps.tile(
                                [128, 512], fp32, tag="op", name=f"po{st}_{oc}"
                            )
                            nc.tensor.matmul(
                                po[:],
                                wo_sb[:, oc, :],
                                ctxT[:, st * 512 : (st + 1) * 512],
                                start=True,
                                stop=True,
                            )
                            engs[k % len(engs)](ob[:, k, :], po[:])
                        dq = nc.sync if (st + j) % 2 == 0 else nc.gpsimd
                        dq.dma_start(
                            out_r[:, j * 2 : j * 2 + 2, st * 512 : (st + 1) * 512],
                            ob[:],
                        )

                    return unit

                def div_unit(b, h, dq):
                    # reciprocal of the den row, DRAM bounce to broadcast
                    # along partitions, scale this head's ctxT rows
                    def unit():
                        pi = b * HPC + h
                        hh = h * DK
                        with nc.allow_low_precision(
                            reason="bf16 softmax reciprocal within tolerance"
                        ):
                            nc.vector.reciprocal(rc128[:, pi, :], dn128[:, pi, :])
                        dq.dma_start(
                            rec_dram[pi * S : (pi + 1) * S].rearrange(
                                "(p c) -> p c", p=128
                            ),
                            rc128[:, pi, :],
                        )
                        R = att_sb.tile(
                            [128, S], bf16, tag="R", bufs=1, name=f"R{pi}"
                        )
                        dq.dma_start(
                            R[hh : hh + DK, :],
                            rec_dram[pi * S : (pi + 1) * S]
                            .rearrange("(o s) -> o s", o=1)
                            .to_broadcast((DK, S)),
                        )
                        sl = slice(b * S, (b + 1) * S)
                        nc.vector.tensor_mul(
                            ctxT[hh : hh + DK, sl],
                            ctxT[hh : hh + DK, sl],
                            R[hh : hh + DK, :],
                        )

                    return unit

                def ctx_units(b, sh, si, expS):
                    # two 16-matmul ctx chains (head 0 / head 1) for the
                    # 512-col stripe st_i, as 8 pump units of 4 MMs + one
                    # evacuation unit. Chains ride the NEXT block's exp.
                    st_i = sh * 2 + si
                    pcs = {}

                    def chain_quarter(h, q):
                        def unit():
                            if q == 0:
                                pcs[h] = ctx_ps.tile(
                                    [DK + 1, 512],
                                    fp32,
                                    tag="pc",
                                    name=f"pc{b}{st_i}{h}",
                                )
                            for tt in range(q * 4, q * 4 + 4):
                                nc.tensor.matmul(
                                    pcs[h][:],
                                    v_sb[
                                        :,
                                        b * 16 + tt,
                                        h * (DK + 1) : (h + 1) * (DK + 1),
                                    ],
                                    expS[:, tt, h * 512 : (h + 1) * 512],
                                    start=(tt == 0),
                                    stop=(tt == 15),
                                )

                        return unit

                    def evacuate():
                        ds0 = b * S + st_i * 512
                        for h in range(HPC):
                            pc = pcs[h]
                            pi = b * HPC + h
                            if h == 0:
                                nc.vector.tensor_copy(
                                    ctxT[0:DK, ds0 : ds0 + 512], pc[0:DK, :]
                                )
                            else:
                                stg = att_sb.tile(
                                    [DK, 512], bf16, tag="stg", bufs=1, name="stg"
                                )
                                nc.vector.tensor_copy(stg[:], pc[0:DK, :])
                                nc.vector.stream_shuffle(
                                    ctxT[DK : 2 * DK, ds0 : ds0 + 512],
                                    stg[:],
                                    mask=list(range(32)),
                                )
                            nc.vector.tensor_copy(
                                den_big[
                                    32 * pi : 32 * pi + 1,
                                    st_i * 512 : (st_i + 1) * 512,
                                ],
                                pc[DK : DK + 1, :],
                            )
                            d0 = pi * S + st_i * 512
                            nc.gpsimd.dma_start(
                                den_dram[d0 : d0 + 512].rearrange(
                                    "(o c) -> o c", o=1
                                ),
                                den_big[
                                    32 * pi : 32 * pi + 1,
                                    st_i * 512 : (st_i + 1) * 512,
                                ],
                            )
                            nc.gpsimd.dma_start(
                                dn128[st_i * 32 : (st_i + 1) * 32, pi, :],
                                den_dram[d0 : d0 + 512].rearrange(
                                    "(p c) -> p c", p=32
                                ),
                            )

                    units = []
                    for q in range(4):
                        units.append(chain_quarter(0, q))
                        units.append(chain_quarter(1, q))
                    units.append(evacuate)
                    return units

                def attn_block(b, sh, si, budget):
                    # both heads' scores into the two banks of one
                    # [128,1024] PSUM tile (concurrent row-group tiles
                    # (0,0)/(64,0)); ONE FD=1024 exp covers both heads.
                    # expS[:, tt, 0:512]=head0, [:, tt, 512:]=head1.
                    expS = att_sb.tile(
                        [128, 16, 1024], bf16, tag="expS", bufs=2,
                        name=f"eS{b}{sh}{si}",
                    )
                    s0 = b * S + sh * 1024 + si * 512
                    for tt in range(16):
                        pump(budget)
                        tb = slice(b * S + tt * 128, b * S + (tt + 1) * 128)
                        ps = sc_ps.tile([128, 1024], fp32, tag="sc", name="psAB")
                        nc.tensor.matmul(
                            ps[:, 0:512], krot[0:DK, tb], qrot[0:DK, s0 : s0 + 512],
                            start=True, stop=True,
                        )
                        nc.tensor.matmul(
                            ps[:, 512:1024],
                            krot[DK:128, tb],
                            qrot[DK:128, s0 : s0 + 512],
                            start=True, stop=True,
                        )
                        nc.scalar.activation(expS[:, tt, :], ps[:], Exp, scale=0.125)
                    return expS

                # pump inventory: V(b0) chains first (xt fully landed by the
                # first block), then batch-1 Q/K chains + their rope, then
                # V(b1); ctx units always jump the queue (extendleft).
                for tt in range(16):
                    work.append(v_chain(tt))
                plain_q1 = qkv_tmp.tile(
                    [128, S], bf16, tag="plain", name="plain_q1"
                )
                plain_k1 = qkv_tmp.tile(
                    [128, S], bf16, tag="plain2", name="plain_k1"
                )
                for st in range(4, 8):
                    work.append(qk_b1_chain(wq_sb, bq_sb, plain_q1, st))
                work.append(lambda: rope(plain_q1, qrot[:, S:NS]))
                for st in range(4, 8):
                    work.append(qk_b1_chain(wk_sb, bk_sb, plain_k1, st))
                work.append(lambda: rope(plain_k1, krot[:, S:NS]))

                blocks = [
                    (b, sh, si)
                    for b in range(B)
                    for sh in range(2)
                    for si in range(2)
                ]
                prev = None
                for bi, (b, sh, si) in enumerate(blocks):
                    if bi == 1:
                        for tt in range(16, 32):
                            work.append(v_chain(tt))
                    if bi == 5:
                        work.append(div_unit(0, 0, nc.gpsimd))
                        work.append(div_unit(0, 1, nc.gpsimd))
                        engs = [nc.vector.tensor_copy]
                        for st in range(4):
                            for j in range(4):
                                work.append(op_quad(st, j, engs))
                    if prev is not None:
                        work.extendleft(reversed(ctx_units(*prev)))
                    expS = attn_block(b, sh, si, budget=1 if bi == 0 else 2)
                    prev = (b, sh, si, expS)

                # drain: last block's ctx, remaining pump work, b1 divs,
                # b1 out-projection
                for u in ctx_units(*prev):
                    u()
                while work:
                    work.popleft()()
                div_unit(1, 0, nc.gpsimd)()
                div_unit(1, 1, nc.sync)()
                engs = [nc.vector.tensor_copy, nc.scalar.copy]
                for st in range(4, 8):
                    for j in range(4):
                        op_quad(st, j, engs)()

    nc.compile()
    return nc


def _rope_tables():
    pos = np.arange(S, dtype=np.float64)
    inv_freq = np.exp(np.arange(0, DK, 2, dtype=np.float64) * (-np.log(10000.0) / DK))
    ang = pos[:, None] * inv_freq[None, :]  # [S, 32]
    cos_t = np.empty((128, S), dtype=np.float32)
    sin_t = np.empty((128, S), dtype=np.float32)
    c = np.cos(ang).astype(np.float32).T  # [32, S]
    s = np.sin(ang).astype(np.float32).T
    for blk in range(4):
        cos_t[blk * 32 : (blk + 1) * 32] = c
        sign = -1.0 if blk % 2 == 0 else 1.0
        sin_t[blk * 32 : (blk + 1) * 32] = sign * s
    return cos_t, sin_t


def _prep_w(w):
    # [1024, 128] column slice -> [128, 8*128] with the 1024-dim split into
    # 8 chunks of 128 on the partition axis (contiguous 2KB DMA lines)
    bf = ml_dtypes.bfloat16
    return np.ascontiguousarray(
        np.asarray(w, dtype=np.float32)
        .reshape(8, 128, 128)
        .transpose(1, 0, 2)
        .reshape(128, 8 * 128)
    ).astype(bf)


def _prep_inputs(inputs, Wq, bq, Wk, bk, Wv, bv, Wo):
    bf = ml_dtypes.bfloat16
    x2 = np.asarray(inputs, dtype=np.float32).reshape(NS, D)
    xt = np.ascontiguousarray(x2.T).astype(bf)
    cos_t, sin_t = _rope_tables()
    cos_b = cos_t.astype(bf)
    sin_b = sin_t.astype(bf)
    in_maps = []
    for c in range(NCORES):
        sl = slice(c * DPC, (c + 1) * DPC)
        in_maps.append(
            {
                "xt": xt,
                "wq": _prep_w(Wq[:, sl]),
                "wk": _prep_w(Wk[:, sl]),
                "wv": _prep_w(Wv[:, sl]),
                "wo": np.ascontiguousarray(Wo[sl, :]).astype(bf),
                "bq": np.ascontiguousarray(bq[sl]).reshape(DPC, 1).astype(np.float32),
                "bk": np.ascontiguousarray(bk[sl]).reshape(DPC, 1).astype(np.float32),
                "bv": np.ascontiguousarray(bv[sl]).reshape(1, DPC).astype(bf),
                "cos": cos_b,
                "sin": sin_b,
            }
        )
    return in_maps


def _get_nc():
    if "nc" not in _cache:
        _cache["nc"] = _build_nc()
    return _cache["nc"]


def run(inputs_dict, trace=False):
    """Build (cached), run on 8 cores, assemble full output. Returns
    (output fp32 [B,S,D], BassKernelResults)."""
    from concourse.bass_utils import run_bass_kernel_spmd

    nc = _get_nc()
    in_maps = _prep_inputs(
        inputs_dict["inputs"],
        inputs_dict["Wq"],
        inputs_dict["bq"],
        inputs_dict["Wk"],
        inputs_dict["bk"],
        inputs_dict["Wv"],
        inputs_dict["bv"],
        inputs_dict["Wo"],
    )
    res = run_bass_kernel_spmd(
        nc, in_maps, core_ids=list(range(NCORES)), trace=trace
    )
    acc = np.zeros((D, NS), dtype=np.float32)
    for r in res.results:
        acc += r["out"].astype(np.float32)
    out = acc.T.reshape(B, S, D) + np.asarray(inputs_dict["bo"], dtype=np.float32)
    return out.astype(np.float32), res


def kernel(**inputs):
    out, _ = run(inputs, trace=False)
    return out
